# revision 22
# baseline (speedup 1.0000x reference)
"""Dynamic per-sample CNN (nn_ConvFunc) Trainium2 Bass kernel — bf16 version.

Reference computation (per sample b):
  cnn_inp = proj_w @ cat(lhs, rhs) + proj_b          # 1x1 conv, [128, 32, 32]
  out     = conv3x3(cnn_inp, W_b) + bias_b           # W_b, bias_b unpacked from question_rep[b]

Sharding: pure data parallel, 8 samples per NeuronCore (batch 64 / 8 cores).

Design:
  - all matmul operands bf16 (tolerance 2e-2 >> bf16's ~3e-3): fp32 matmul
    streams at half rate on the trn2 PE, so bf16 halves PE time (90112
    columns -> 37.5us @ 2.4GHz) and halves HBM traffic.
  - per-sample inputs packed host-side into one [128, 3474] row
    ([qw | consts | xl_h0 | xr_h0 | xl_h1 | xr_h1]); loads split x-part /
    w-part and issued up-front on the sync HWDGE ring in exact consumption
    order (x0, x1, w0, x2, w1, ...) — each DMA_DIRECT2D issue costs ~650ns
    of sequencer time and completion semaphores lag the last byte by ~2us,
    so few big DMAs in consumption order win.
  - proj weights + biases ride inside sample 0's px row (biases as fp32
    bit-pattern pairs of bf16 columns, bitcast on device): no separate
    small-descriptor const DMA (a 36B/partition DMA starved behind the px
    stream cost 7us in an earlier rev).
  - dummy warmup matmuls fill the DMA ramp and known early stalls so the PE
    HAM clock gate lifts at ~11us and never re-throttles (idle >3.4us would
    drop the PE clock 2.4 -> 1.2 GHz).
  - PSUM eviction split across DVE (proj/conv h0) and ACT (proj/conv h1);
    output stored bf16 (host upcasts). Last sample runs h-outer with its
    second half column-chunked so the final evict+store tail is ~256 cols.
"""

import numpy as np
import ml_dtypes

import concourse.bass as bass
import concourse.mybir as mybir
from concourse import bacc
from concourse.tile import TileContext
from concourse.bass_utils import run_bass_kernel_spmd

# Problem shapes (hardcoded per contract)
B = 64
DIM = 128
H = W = 32
K = 3
KK = K * K
HW = H * W             # 1024
WDIM = DIM * DIM * KK  # 147456
NCORES = 8
SPC = B // NCORES      # samples per core
HP, WP = H + 2, W + 2  # padded 34x34
HALF = HW // 2         # 512 columns per PSUM bank
QUART = HALF // 2      # 256-col tail chunks
HROWS = H // 2         # 16 output rows per half
QROWS = HROWS // 2     # 8 rows per tail chunk
QWC = KK * DIM         # 1152 conv-weight cols per sample
CSTC = 2 * DIM + 2 * (SPC + 1)  # pw cols + bitcast fp32 bias cols (274)
XB = QWC + CSTC        # x columns start here
PXC = XB + 2 * HW      # 3474 packed cols: [qw | cst | xlh0 | xrh0 | xlh1 | xrh1]

FP = mybir.dt.float32
BF = mybir.dt.bfloat16
BF_NP = ml_dtypes.bfloat16

_BUILT = {}


def build_nc():
    nc = bacc.Bacc("TRN2", target_bir_lowering=False, debug=False,
                   num_devices=NCORES)

    px = nc.declare_dram_parameter("px", [SPC, DIM, PXC], BF, isOutput=False)
    out = nc.declare_dram_parameter("out", [SPC, DIM, HW], BF, isOutput=True)

    with TileContext(nc) as tc:
        with (
            tc.tile_pool(name="const", bufs=1) as cpool,
            tc.tile_pool(name="pxpool", bufs=SPC) as pxpool,
            tc.tile_pool(name="xppool", bufs=4) as xppool,
            tc.tile_pool(name="opool", bufs=4) as opool,
            tc.tile_pool(name="pp_pool", bufs=2, space="PSUM") as pp_pool,
            tc.tile_pool(name="pc_pool", bufs=5, space="PSUM") as pc_pool,
        ):
            warm = cpool.tile([DIM, HALF], BF)
            # gates the PE warmup matmuls; gpsimd reaches its first user
            # instruction earliest and is otherwise idle
            nc.gpsimd.memset(warm[:], 0.0)

            px_sb = [pxpool.tile([DIM, PXC], BF, tag="px", name=f"px{s}")
                     for s in range(SPC)]

            # ---- all loads up-front on the sync ring, consumption order ---
            def load_x(s):
                # sample 0 carries the consts: load them in the same DMA
                lo = QWC if s == 0 else XB
                nc.sync.dma_start(out=px_sb[s][:, lo:PXC],
                                  in_=px[s, :, lo:PXC])

            def load_w(s):
                nc.sync.dma_start(out=px_sb[s][:, 0:QWC], in_=px[s, :, 0:QWC])

            # consumption order: conv(s) needs w(s) right after proj(s+1)
            # needs x(s+1), so pair them w-then-x. Sample 0 splits at the
            # pixel-half boundary so proj(0) h0 gates on a smaller DMA;
            # sample 7 loads its whole row in one DMA (qw+dead cst+x) to
            # keep the total issue count unchanged (issue slots cost ~650ns
            # each and delay every later gate — the v5 lesson).
            nc.sync.dma_start(out=px_sb[0][:, QWC:XB + HW],
                              in_=px[0, :, QWC:XB + HW])
            nc.sync.dma_start(out=px_sb[0][:, XB + HW:PXC],
                              in_=px[0, :, XB + HW:PXC])
            load_x(1)
            for s in range(SPC - 3):
                load_w(s)
                load_x(s + 2)
            load_w(SPC - 3)
            nc.sync.dma_start(out=px_sb[SPC - 1][:], in_=px[SPC - 1])
            load_w(SPC - 2)

            # ---- PE warmup: lift the HAM clock gate during the DMA ramp ---
            # the warmup accumulator shares the proj PSUM pool: warmups and
            # proj matmuls are all PE-ordered, so the WAR costs nothing
            wmb = pp_pool.tile([DIM, HALF], FP, tag="pp")

            def warmup(n):
                # N=256 quanta: finer granularity wastes less PE time when
                # the gating DMA completes mid-warmup
                for _ in range(n):
                    nc.tensor.matmul(wmb[:, 0:QUART], lhsT=warm[:, 0:DIM],
                                     rhs=warm[:, 0:QUART],
                                     start=True, stop=True)

            warmup(18)

            pw0 = px_sb[0][:, QWC:QWC + DIM]
            pw1 = px_sb[0][:, QWC + DIM:QWC + 2 * DIM]

            def qb_ap(s):
                o = QWC + 2 * DIM + 2 * s
                return px_sb[0][:, o:o + 2].bitcast(FP)

            pb_ap = px_sb[0][:, XB - 2:XB].bitcast(FP)

            def proj(s):
                xp = xppool.tile([DIM, HP, WP], BF, tag="xp")
                if s < 4:
                    # borders only need zeroing once per pool buffer; the
                    # interior is fully rewritten every rotation
                    nc.vector.memset(xp[:, 0:1, :], 0.0)
                    nc.vector.memset(xp[:, HP - 1:HP, :], 0.0)
                    nc.vector.memset(xp[:, 1:HP - 1, 0:1], 0.0)
                    nc.vector.memset(xp[:, 1:HP - 1, WP - 1:WP], 0.0)
                for h in range(2):
                    ppt = pp_pool.tile([DIM, HALF], FP, tag="pp")
                    nc.tensor.matmul(ppt[:], lhsT=pw0,
                                     rhs=px_sb[s][:, XB + HW * h:
                                                  XB + HW * h + HALF],
                                     start=True, stop=False)
                    nc.tensor.matmul(ppt[:], lhsT=pw1,
                                     rhs=px_sb[s][:, XB + HW * h + HALF:
                                                  XB + HW * (h + 1)],
                                     start=False, stop=True)
                    dst = xp[:, 1 + HROWS * h:1 + HROWS * (h + 1), 1:1 + W]
                    src = ppt[:].rearrange("p (a b) -> p a b", b=W)
                    if h == 0:
                        # h0 on DVE, h1 on ACT: parallel evictions unblock
                        # the dependent conv two engine-ops sooner
                        nc.vector.tensor_scalar_add(dst, src, pb_ap)
                    else:
                        nc.scalar.activation(
                            dst, src,
                            mybir.ActivationFunctionType.Identity,
                            bias=pb_ap,
                        )
                return xp

            def wtap(s, t):
                return px_sb[s][:, t * DIM:(t + 1) * DIM]

            def conv(s, xp):
                o_sb = opool.tile([DIM, HW], BF, tag="o")
                pct0 = pc_pool.tile([DIM, HALF], FP, tag="pc")
                pct1 = pc_pool.tile([DIM, HALF], FP, tag="pc")
                pcts = [pct0, pct1]
                qb = qb_ap(s)
                # tap-outer: consecutive matmuls share the stationary weights
                for t in range(KK):
                    kh, kw = divmod(t, K)
                    for h in range(2):
                        nc.tensor.matmul(
                            pcts[h][:],
                            lhsT=wtap(s, t),
                            rhs=xp[:, HROWS * h + kh:HROWS * (h + 1) + kh,
                                   kw:kw + W],
                            start=(t == 0), stop=(t == KK - 1))
                # evict half 0 on DVE, half 1 on ACT; bf16 out halves traffic
                nc.vector.tensor_scalar_add(o_sb[:, 0:HALF], pct0[:], qb)
                nc.scalar.activation(
                    o_sb[:, HALF:HW], pct1[:],
                    mybir.ActivationFunctionType.Identity, bias=qb)
                nc.scalar.dma_start(out=out[s], in_=o_sb[:])

            def conv_last(s, xp):
                # h-outer + column-chunked second half: each chunk evicts and
                # stores while later chunks' taps still run -> short tail
                o_sb = opool.tile([DIM, HW], BF, tag="o")
                qb = qb_ap(s)
                pct0 = pc_pool.tile([DIM, HALF], FP, tag="pc")
                for t in range(KK):
                    kh, kw = divmod(t, K)
                    nc.tensor.matmul(
                        pct0[:], lhsT=wtap(s, t),
                        rhs=xp[:, kh:HROWS + kh, kw:kw + W],
                        start=(t == 0), stop=(t == KK - 1))
                nc.vector.tensor_scalar_add(o_sb[:, 0:HALF], pct0[:], qb)
                nc.scalar.dma_start(out=out[s, :, 0:HALF],
                                    in_=o_sb[:, 0:HALF])
                for c in range(2):
                    pcq = pc_pool.tile([DIM, HALF], FP, tag="pc")
                    r0 = HROWS + QROWS * c
                    c0 = HALF + QUART * c
                    for t in range(KK):
                        kh, kw = divmod(t, K)
                        nc.tensor.matmul(
                            pcq[:, 0:QUART], lhsT=wtap(s, t),
                            rhs=xp[:, r0 + kh:r0 + QROWS + kh, kw:kw + W],
                            start=(t == 0), stop=(t == KK - 1))
                    nc.vector.tensor_scalar_add(
                        o_sb[:, c0:c0 + QUART], pcq[:, 0:QUART], qb)
                    nc.scalar.dma_start(out=out[s, :, c0:c0 + QUART],
                                        in_=o_sb[:, c0:c0 + QUART])

            # software pipeline: proj(s) ahead of conv(s-1) keeps PE dense;
            # warmup matmuls fill the known early DMA-ramp stalls
            prev = None
            for s in range(SPC):
                if s == 1:
                    warmup(2)
                xp = proj(s)
                if s == 1:
                    warmup(2)
                if prev is not None:
                    if prev[0] == SPC - 1:
                        conv_last(*prev)
                    else:
                        conv(*prev)
                prev = (s, xp)
            conv_last(*prev)

    nc.compile()
    return nc


def _prep(question_rep, lhs_rep, rhs_rep, proj_w, proj_b):
    """Host-side shard + layout prep (reshape/transpose + bf16 cast)."""
    qr = np.ascontiguousarray(question_rep, dtype=np.float32)
    # conv weights: [B, o, i, kh, kw] -> [B, i, (kh kw), o] so each tap is a
    # ready lhsT [i, o] block
    qw = qr[:, :WDIM].reshape(B, DIM, DIM, K, K).transpose(0, 2, 3, 4, 1)
    qw = np.ascontiguousarray(qw).reshape(B, DIM, QWC)
    qb = np.ascontiguousarray(qr[:, WDIM:])             # [B, 128]
    xl = np.asarray(lhs_rep, dtype=np.float32).reshape(B, DIM, HW)
    xr = np.asarray(rhs_rep, dtype=np.float32).reshape(B, DIM, HW)
    pwt = np.asarray(proj_w, dtype=np.float32).T        # [256, 128]
    pw_h = np.concatenate([pwt[:DIM], pwt[DIM:]], axis=1).astype(BF_NP)
    pb = np.asarray(proj_b, dtype=np.float32).reshape(DIM, 1)

    in_maps = []
    for c in range(NCORES):
        sl = slice(c * SPC, (c + 1) * SPC)
        # biases ride as fp32 bit-patterns in bf16 columns (device bitcasts);
        # the whole const block replicates into every sample's px row so it
        # arrives inside sample 0's big first DMA (no tiny-descriptor DMA)
        biasm = np.concatenate([qb[sl].T, pb], axis=1).astype(np.float32)
        bias_bf = np.ascontiguousarray(biasm).view(np.uint16).view(BF_NP)
        cstm = np.concatenate([pw_h, bias_bf], axis=1)  # [128, 274] bf16
        cst8 = np.broadcast_to(cstm[None], (SPC, DIM, CSTC))
        pxm = np.concatenate(
            [qw[sl].astype(BF_NP), cst8,
             xl[sl, :, :HALF].astype(BF_NP), xr[sl, :, :HALF].astype(BF_NP),
             xl[sl, :, HALF:].astype(BF_NP), xr[sl, :, HALF:].astype(BF_NP)],
            axis=2)  # [SPC, 128, 3474]
        in_maps.append({
            "px": np.ascontiguousarray(pxm),
        })
    return in_maps


def kernel(question_rep, lhs_rep, rhs_rep, proj_w, proj_b, _run_kwargs=None):
    if "nc" not in _BUILT:
        _BUILT["nc"] = build_nc()
    nc = _BUILT["nc"]
    in_maps = _prep(question_rep, lhs_rep, rhs_rep, proj_w, proj_b)
    res = run_bass_kernel_spmd(nc, in_maps, core_ids=list(range(NCORES)),
                               **(_run_kwargs or {}))
    out = np.concatenate([np.asarray(res.results[c]["out"])
                          for c in range(NCORES)], axis=0)
    if _run_kwargs is not None:
        _BUILT["last_result"] = res
    return out.astype(np.float32).reshape(B, DIM, H, W)


if __name__ == "__main__":
    rng = np.random.default_rng(0)
    inputs = {
        "question_rep": rng.standard_normal((B, WDIM + DIM), dtype=np.float32) * 0.05,
        "lhs_rep": rng.standard_normal((B, DIM, H, W), dtype=np.float32),
        "rhs_rep": rng.standard_normal((B, DIM, H, W), dtype=np.float32),
        "proj_w": rng.standard_normal((DIM, 2 * DIM), dtype=np.float32),
        "proj_b": rng.standard_normal((DIM,), dtype=np.float32) * 0.01,
    }
    out = kernel(**inputs)
    print("ran, out shape:", out.shape)


# revision 25
# speedup vs baseline: 1.0032x; 1.0032x over previous
"""Dynamic per-sample CNN (nn_ConvFunc) Trainium2 Bass kernel — bf16 version.

Reference computation (per sample b):
  cnn_inp = proj_w @ cat(lhs, rhs) + proj_b          # 1x1 conv, [128, 32, 32]
  out     = conv3x3(cnn_inp, W_b) + bias_b           # W_b, bias_b unpacked from question_rep[b]

Sharding: pure data parallel, 8 samples per NeuronCore (batch 64 / 8 cores).

Design:
  - all matmul operands bf16 (tolerance 2e-2 >> bf16's ~3e-3): fp32 matmul
    streams at half rate on the trn2 PE, so bf16 halves PE time (90112
    columns -> 37.5us @ 2.4GHz) and halves HBM traffic.
  - per-sample inputs packed host-side into one [128, 3474] row
    ([qw | consts | xl_h0 | xr_h0 | xl_h1 | xr_h1]); loads split x-part /
    w-part and issued up-front on the sync HWDGE ring in exact consumption
    order (x0, x1, w0, x2, w1, ...) — each DMA_DIRECT2D issue costs ~650ns
    of sequencer time and completion semaphores lag the last byte by ~2us,
    so few big DMAs in consumption order win.
  - proj weights + biases ride inside sample 0's px row (biases as fp32
    bit-pattern pairs of bf16 columns, bitcast on device): no separate
    small-descriptor const DMA (a 36B/partition DMA starved behind the px
    stream cost 7us in an earlier rev).
  - dummy warmup matmuls fill the DMA ramp and known early stalls so the PE
    HAM clock gate lifts at ~11us and never re-throttles (idle >3.4us would
    drop the PE clock 2.4 -> 1.2 GHz).
  - PSUM eviction split across DVE (proj/conv h0) and ACT (proj/conv h1);
    output stored bf16 (host upcasts). Last sample runs h-outer with its
    second half column-chunked so the final evict+store tail is ~256 cols.
"""

import numpy as np
import ml_dtypes

import concourse.bass as bass
import concourse.mybir as mybir
from concourse import bacc
from concourse.tile import TileContext
from concourse.bass_utils import run_bass_kernel_spmd

# Problem shapes (hardcoded per contract)
B = 64
DIM = 128
H = W = 32
K = 3
KK = K * K
HW = H * W             # 1024
WDIM = DIM * DIM * KK  # 147456
NCORES = 8
SPC = B // NCORES      # samples per core
HP, WP = H + 2, W + 2  # padded 34x34
HALF = HW // 2         # 512 columns per PSUM bank
QUART = HALF // 2      # 256-col tail chunks
HROWS = H // 2         # 16 output rows per half
QROWS = HROWS // 2     # 8 rows per tail chunk
QWC = KK * DIM         # 1152 conv-weight cols per sample
CSTC = 2 * DIM + 2 * (SPC + 1)  # pw cols + bitcast fp32 bias cols (274)
XB = QWC + CSTC        # x columns start here
PXC = XB + 2 * HW      # 3474 packed cols: [qw | cst | xlh0 | xrh0 | xlh1 | xrh1]

FP = mybir.dt.float32
BF = mybir.dt.bfloat16
BF_NP = ml_dtypes.bfloat16

_BUILT = {}


def build_nc():
    nc = bacc.Bacc("TRN2", target_bir_lowering=False, debug=False,
                   num_devices=NCORES)

    px = nc.declare_dram_parameter("px", [SPC, DIM, PXC], BF, isOutput=False)
    out = nc.declare_dram_parameter("out", [SPC, DIM, HW], BF, isOutput=True)

    with TileContext(nc) as tc:
        with (
            tc.tile_pool(name="const", bufs=1) as cpool,
            tc.tile_pool(name="pxpool", bufs=SPC) as pxpool,
            tc.tile_pool(name="xppool", bufs=4) as xppool,
            tc.tile_pool(name="opool", bufs=4) as opool,
            tc.tile_pool(name="pp_pool", bufs=3, space="PSUM") as pp_pool,
            tc.tile_pool(name="pc_pool", bufs=5, space="PSUM") as pc_pool,
        ):
            warm = cpool.tile([DIM, HALF], BF)
            # gates the PE warmup matmuls; gpsimd reaches its first user
            # instruction earliest and is otherwise idle
            nc.gpsimd.memset(warm[:], 0.0)

            px_sb = [pxpool.tile([DIM, PXC], BF, tag="px", name=f"px{s}")
                     for s in range(SPC)]

            # ---- all loads up-front on the sync ring, consumption order ---
            def load_x(s):
                # sample 0 carries the consts: load them in the same DMA
                lo = QWC if s == 0 else XB
                nc.sync.dma_start(out=px_sb[s][:, lo:PXC],
                                  in_=px[s, :, lo:PXC])

            def load_w(s):
                nc.sync.dma_start(out=px_sb[s][:, 0:QWC], in_=px[s, :, 0:QWC])

            # consumption order: conv(s) needs w(s) right after proj(s+1)
            # needs x(s+1), so pair them w-then-x. Sample 0 splits at the
            # pixel-half boundary so proj(0) h0 gates on a smaller DMA;
            # sample 7 loads its whole row in one DMA (qw+dead cst+x) to
            # keep the total issue count unchanged (issue slots cost ~650ns
            # each and delay every later gate — the v5 lesson).
            nc.sync.dma_start(out=px_sb[0][:, QWC:XB + HW],
                              in_=px[0, :, QWC:XB + HW])
            nc.sync.dma_start(out=px_sb[0][:, XB + HW:PXC],
                              in_=px[0, :, XB + HW:PXC])
            # w0 rides the otherwise-idle ACT HWDGE ring: it transfers
            # concurrently with the sync stream, pulling the conv(0) and
            # x1 gates (the delivery-bound window) ~0.5us earlier
            nc.scalar.dma_start(out=px_sb[0][:, 0:QWC], in_=px[0, :, 0:QWC])
            load_x(1)
            for s in range(SPC - 3):
                if s > 0:
                    load_w(s)
                load_x(s + 2)
            load_w(SPC - 3)
            nc.sync.dma_start(out=px_sb[SPC - 1][:], in_=px[SPC - 1])
            load_w(SPC - 2)

            # ---- PE warmup: lift the HAM clock gate during the DMA ramp ---
            # the warmup accumulator shares the proj PSUM pool: warmups and
            # proj matmuls are all PE-ordered, so the WAR costs nothing
            wmb = pp_pool.tile([DIM, HALF], FP, tag="pp")

            def warmup(n):
                # N=256 quanta: finer granularity wastes less PE time when
                # the gating DMA completes mid-warmup
                for _ in range(n):
                    nc.tensor.matmul(wmb[:, 0:QUART], lhsT=warm[:, 0:DIM],
                                     rhs=warm[:, 0:QUART],
                                     start=True, stop=True)

            warmup(16)

            pw0 = px_sb[0][:, QWC:QWC + DIM]
            pw1 = px_sb[0][:, QWC + DIM:QWC + 2 * DIM]

            def qb_ap(s):
                o = QWC + 2 * DIM + 2 * s
                return px_sb[0][:, o:o + 2].bitcast(FP)

            pb_ap = px_sb[0][:, XB - 2:XB].bitcast(FP)

            def proj(s):
                xp = xppool.tile([DIM, HP, WP], BF, tag="xp")
                if s < 4:
                    # borders only need zeroing once per pool buffer; the
                    # interior is fully rewritten every rotation
                    nc.vector.memset(xp[:, 0:1, :], 0.0)
                    nc.vector.memset(xp[:, HP - 1:HP, :], 0.0)
                    nc.vector.memset(xp[:, 1:HP - 1, 0:1], 0.0)
                    nc.vector.memset(xp[:, 1:HP - 1, WP - 1:WP], 0.0)
                for h in range(2):
                    ppt = pp_pool.tile([DIM, HALF], FP, tag="pp")
                    nc.tensor.matmul(ppt[:], lhsT=pw0,
                                     rhs=px_sb[s][:, XB + HW * h:
                                                  XB + HW * h + HALF],
                                     start=True, stop=False)
                    nc.tensor.matmul(ppt[:], lhsT=pw1,
                                     rhs=px_sb[s][:, XB + HW * h + HALF:
                                                  XB + HW * (h + 1)],
                                     start=False, stop=True)
                    dst = xp[:, 1 + HROWS * h:1 + HROWS * (h + 1), 1:1 + W]
                    src = ppt[:].rearrange("p (a b) -> p a b", b=W)
                    if h == 0:
                        # h0 on DVE, h1 on ACT: parallel evictions unblock
                        # the dependent conv two engine-ops sooner
                        nc.vector.tensor_scalar_add(dst, src, pb_ap)
                    else:
                        nc.scalar.activation(
                            dst, src,
                            mybir.ActivationFunctionType.Identity,
                            bias=pb_ap,
                        )
                return xp

            def wtap(s, t):
                return px_sb[s][:, t * DIM:(t + 1) * DIM]

            def conv(s, xp):
                o_sb = opool.tile([DIM, HW], BF, tag="o")
                pct0 = pc_pool.tile([DIM, HALF], FP, tag="pc")
                pct1 = pc_pool.tile([DIM, HALF], FP, tag="pc")
                pcts = [pct0, pct1]
                qb = qb_ap(s)
                # tap-outer: consecutive matmuls share the stationary weights
                for t in range(KK):
                    kh, kw = divmod(t, K)
                    for h in range(2):
                        nc.tensor.matmul(
                            pcts[h][:],
                            lhsT=wtap(s, t),
                            rhs=xp[:, HROWS * h + kh:HROWS * (h + 1) + kh,
                                   kw:kw + W],
                            start=(t == 0), stop=(t == KK - 1))
                # evict half 0 on DVE, half 1 on ACT; bf16 out halves traffic
                nc.vector.tensor_scalar_add(o_sb[:, 0:HALF], pct0[:], qb)
                nc.scalar.activation(
                    o_sb[:, HALF:HW], pct1[:],
                    mybir.ActivationFunctionType.Identity, bias=qb)
                nc.scalar.dma_start(out=out[s], in_=o_sb[:])

            def conv_last(s, xp):
                # h-outer + column-chunked second half: each chunk evicts and
                # stores while later chunks' taps still run -> short tail
                o_sb = opool.tile([DIM, HW], BF, tag="o")
                qb = qb_ap(s)
                pct0 = pc_pool.tile([DIM, HALF], FP, tag="pc")
                for t in range(KK):
                    kh, kw = divmod(t, K)
                    nc.tensor.matmul(
                        pct0[:], lhsT=wtap(s, t),
                        rhs=xp[:, kh:HROWS + kh, kw:kw + W],
                        start=(t == 0), stop=(t == KK - 1))
                nc.vector.tensor_scalar_add(o_sb[:, 0:HALF], pct0[:], qb)
                nc.scalar.dma_start(out=out[s, :, 0:HALF],
                                    in_=o_sb[:, 0:HALF])
                for c in range(2):
                    pcq = pc_pool.tile([DIM, HALF], FP, tag="pc")
                    r0 = HROWS + QROWS * c
                    c0 = HALF + QUART * c
                    for t in range(KK):
                        kh, kw = divmod(t, K)
                        nc.tensor.matmul(
                            pcq[:, 0:QUART], lhsT=wtap(s, t),
                            rhs=xp[:, r0 + kh:r0 + QROWS + kh, kw:kw + W],
                            start=(t == 0), stop=(t == KK - 1))
                    nc.vector.tensor_scalar_add(
                        o_sb[:, c0:c0 + QUART], pcq[:, 0:QUART], qb)
                    nc.scalar.dma_start(out=out[s, :, c0:c0 + QUART],
                                        in_=o_sb[:, c0:c0 + QUART])

            # software pipeline: proj(s) ahead of conv(s-1) keeps PE dense;
            # warmup matmuls fill the known early DMA-ramp stalls
            prev = None
            for s in range(SPC):
                if s == 1:
                    warmup(2)
                xp = proj(s)
                if s == 1:
                    warmup(2)
                if prev is not None:
                    if prev[0] == SPC - 1:
                        conv_last(*prev)
                    else:
                        conv(*prev)
                prev = (s, xp)
            conv_last(*prev)

    nc.compile()
    return nc


def _prep(question_rep, lhs_rep, rhs_rep, proj_w, proj_b):
    """Host-side shard + layout prep (reshape/transpose + bf16 cast)."""
    qr = np.ascontiguousarray(question_rep, dtype=np.float32)
    # conv weights: [B, o, i, kh, kw] -> [B, i, (kh kw), o] so each tap is a
    # ready lhsT [i, o] block
    qw = qr[:, :WDIM].reshape(B, DIM, DIM, K, K).transpose(0, 2, 3, 4, 1)
    qw = np.ascontiguousarray(qw).reshape(B, DIM, QWC)
    qb = np.ascontiguousarray(qr[:, WDIM:])             # [B, 128]
    xl = np.asarray(lhs_rep, dtype=np.float32).reshape(B, DIM, HW)
    xr = np.asarray(rhs_rep, dtype=np.float32).reshape(B, DIM, HW)
    pwt = np.asarray(proj_w, dtype=np.float32).T        # [256, 128]
    pw_h = np.concatenate([pwt[:DIM], pwt[DIM:]], axis=1).astype(BF_NP)
    pb = np.asarray(proj_b, dtype=np.float32).reshape(DIM, 1)

    in_maps = []
    for c in range(NCORES):
        sl = slice(c * SPC, (c + 1) * SPC)
        # biases ride as fp32 bit-patterns in bf16 columns (device bitcasts);
        # the whole const block replicates into every sample's px row so it
        # arrives inside sample 0's big first DMA (no tiny-descriptor DMA)
        biasm = np.concatenate([qb[sl].T, pb], axis=1).astype(np.float32)
        bias_bf = np.ascontiguousarray(biasm).view(np.uint16).view(BF_NP)
        cstm = np.concatenate([pw_h, bias_bf], axis=1)  # [128, 274] bf16
        cst8 = np.broadcast_to(cstm[None], (SPC, DIM, CSTC))
        pxm = np.concatenate(
            [qw[sl].astype(BF_NP), cst8,
             xl[sl, :, :HALF].astype(BF_NP), xr[sl, :, :HALF].astype(BF_NP),
             xl[sl, :, HALF:].astype(BF_NP), xr[sl, :, HALF:].astype(BF_NP)],
            axis=2)  # [SPC, 128, 3474]
        in_maps.append({
            "px": np.ascontiguousarray(pxm),
        })
    return in_maps


def kernel(question_rep, lhs_rep, rhs_rep, proj_w, proj_b, _run_kwargs=None):
    if "nc" not in _BUILT:
        _BUILT["nc"] = build_nc()
    nc = _BUILT["nc"]
    in_maps = _prep(question_rep, lhs_rep, rhs_rep, proj_w, proj_b)
    res = run_bass_kernel_spmd(nc, in_maps, core_ids=list(range(NCORES)),
                               **(_run_kwargs or {}))
    out = np.concatenate([np.asarray(res.results[c]["out"])
                          for c in range(NCORES)], axis=0)
    if _run_kwargs is not None:
        _BUILT["last_result"] = res
    return out.astype(np.float32).reshape(B, DIM, H, W)


if __name__ == "__main__":
    rng = np.random.default_rng(0)
    inputs = {
        "question_rep": rng.standard_normal((B, WDIM + DIM), dtype=np.float32) * 0.05,
        "lhs_rep": rng.standard_normal((B, DIM, H, W), dtype=np.float32),
        "rhs_rep": rng.standard_normal((B, DIM, H, W), dtype=np.float32),
        "proj_w": rng.standard_normal((DIM, 2 * DIM), dtype=np.float32),
        "proj_b": rng.standard_normal((DIM,), dtype=np.float32) * 0.01,
    }
    out = kernel(**inputs)
    print("ran, out shape:", out.shape)


# revision 26
# speedup vs baseline: 1.0134x; 1.0101x over previous
"""Dynamic per-sample CNN (nn_ConvFunc) Trainium2 Bass kernel — bf16 version.

Reference computation (per sample b):
  cnn_inp = proj_w @ cat(lhs, rhs) + proj_b          # 1x1 conv, [128, 32, 32]
  out     = conv3x3(cnn_inp, W_b) + bias_b           # W_b, bias_b unpacked from question_rep[b]

Sharding: pure data parallel, 8 samples per NeuronCore (batch 64 / 8 cores).

Design:
  - all matmul operands bf16 (tolerance 2e-2 >> bf16's ~3e-3): fp32 matmul
    streams at half rate on the trn2 PE, so bf16 halves PE time (90112
    columns -> 37.5us @ 2.4GHz) and halves HBM traffic.
  - per-sample inputs packed host-side into one [128, 3474] row
    ([qw | consts | xl_h0 | xr_h0 | xl_h1 | xr_h1]); loads split x-part /
    w-part and issued up-front on the sync HWDGE ring in exact consumption
    order (x0, x1, w0, x2, w1, ...) — each DMA_DIRECT2D issue costs ~650ns
    of sequencer time and completion semaphores lag the last byte by ~2us,
    so few big DMAs in consumption order win.
  - proj weights + biases ride inside sample 0's px row (biases as fp32
    bit-pattern pairs of bf16 columns, bitcast on device): no separate
    small-descriptor const DMA (a 36B/partition DMA starved behind the px
    stream cost 7us in an earlier rev).
  - dummy warmup matmuls fill the DMA ramp and known early stalls so the PE
    HAM clock gate lifts at ~11us and never re-throttles (idle >3.4us would
    drop the PE clock 2.4 -> 1.2 GHz).
  - PSUM eviction split across DVE (proj/conv h0) and ACT (proj/conv h1);
    output stored bf16 (host upcasts). Last sample runs h-outer with its
    second half column-chunked so the final evict+store tail is ~256 cols.
"""

import numpy as np
import ml_dtypes

import concourse.bass as bass
import concourse.mybir as mybir
from concourse import bacc
from concourse.tile import TileContext
from concourse.bass_utils import run_bass_kernel_spmd

# Problem shapes (hardcoded per contract)
B = 64
DIM = 128
H = W = 32
K = 3
KK = K * K
HW = H * W             # 1024
WDIM = DIM * DIM * KK  # 147456
NCORES = 8
SPC = B // NCORES      # samples per core
HP, WP = H + 2, W + 2  # padded 34x34
HALF = HW // 2         # 512 columns per PSUM bank
QUART = HALF // 2      # 256-col tail chunks
HROWS = H // 2         # 16 output rows per half
QROWS = HROWS // 2     # 8 rows per tail chunk
QWC = KK * DIM         # 1152 conv-weight cols per sample
CSTC = 2 * DIM + 2 * (SPC + 1)  # pw cols + bitcast fp32 bias cols (274)
XB = QWC + CSTC        # x columns start here
PXC = XB + 2 * HW      # 3474 packed cols: [qw | cst | xlh0 | xrh0 | xlh1 | xrh1]

FP = mybir.dt.float32
BF = mybir.dt.bfloat16
BF_NP = ml_dtypes.bfloat16

_BUILT = {}


def build_nc():
    nc = bacc.Bacc("TRN2", target_bir_lowering=False, debug=False,
                   num_devices=NCORES)

    px = nc.declare_dram_parameter("px", [SPC, DIM, PXC], BF, isOutput=False)
    out = nc.declare_dram_parameter("out", [SPC, DIM, HW], BF, isOutput=True)

    with TileContext(nc) as tc:
        with (
            tc.tile_pool(name="const", bufs=1) as cpool,
            tc.tile_pool(name="pxpool", bufs=SPC) as pxpool,
            tc.tile_pool(name="xppool", bufs=4) as xppool,
            tc.tile_pool(name="opool", bufs=4) as opool,
            tc.tile_pool(name="pp_pool", bufs=3, space="PSUM") as pp_pool,
            tc.tile_pool(name="pc_pool", bufs=5, space="PSUM") as pc_pool,
        ):
            warm = cpool.tile([DIM, HALF], BF)
            # gates the PE warmup matmuls; gpsimd reaches its first user
            # instruction earliest and is otherwise idle
            nc.gpsimd.memset(warm[:], 0.0)

            px_sb = [pxpool.tile([DIM, PXC], BF, tag="px", name=f"px{s}")
                     for s in range(SPC)]

            # ---- all loads up-front on the sync ring, consumption order ---
            def load_x(s):
                # sample 0 carries the consts: load them in the same DMA
                lo = QWC if s == 0 else XB
                nc.sync.dma_start(out=px_sb[s][:, lo:PXC],
                                  in_=px[s, :, lo:PXC])

            def load_w(s):
                nc.sync.dma_start(out=px_sb[s][:, 0:QWC], in_=px[s, :, 0:QWC])

            # consumption order: conv(s) needs w(s) right after proj(s+1)
            # needs x(s+1), so pair them w-then-x. Sample 0 splits at the
            # pixel-half boundary so proj(0) h0 gates on a smaller DMA;
            # sample 7 loads its whole row in one DMA (qw+dead cst+x) to
            # keep the total issue count unchanged (issue slots cost ~650ns
            # each and delay every later gate — the v5 lesson).
            nc.sync.dma_start(out=px_sb[0][:, QWC:XB + HW],
                              in_=px[0, :, QWC:XB + HW])
            nc.sync.dma_start(out=px_sb[0][:, XB + HW:PXC],
                              in_=px[0, :, XB + HW:PXC])
            # w0 rides the otherwise-idle ACT HWDGE ring: it transfers
            # concurrently with the sync stream, pulling the conv(0) and
            # x1 gates (the delivery-bound window) ~0.5us earlier
            nc.scalar.dma_start(out=px_sb[0][:, 0:QWC], in_=px[0, :, 0:QWC])
            load_x(1)
            for s in range(SPC - 3):
                if s > 0:
                    load_w(s)
                load_x(s + 2)
            load_w(SPC - 3)
            nc.sync.dma_start(out=px_sb[SPC - 1][:], in_=px[SPC - 1])
            load_w(SPC - 2)

            # ---- PE warmup: lift the HAM clock gate during the DMA ramp ---
            # the warmup accumulator shares the proj PSUM pool: warmups and
            # proj matmuls are all PE-ordered, so the WAR costs nothing
            wmb = pp_pool.tile([DIM, HALF], FP, tag="pp")

            def warmup(n):
                # N=256 quanta: finer granularity wastes less PE time when
                # the gating DMA completes mid-warmup
                for _ in range(n):
                    nc.tensor.matmul(wmb[:, 0:QUART], lhsT=warm[:, 0:DIM],
                                     rhs=warm[:, 0:QUART],
                                     start=True, stop=True)

            warmup(16)

            pw0 = px_sb[0][:, QWC:QWC + DIM]
            pw1 = px_sb[0][:, QWC + DIM:QWC + 2 * DIM]

            def qb_ap(s):
                o = QWC + 2 * DIM + 2 * s
                return px_sb[0][:, o:o + 2].bitcast(FP)

            pb_ap = px_sb[0][:, XB - 2:XB].bitcast(FP)

            def proj(s):
                xp = xppool.tile([DIM, HP, WP], BF, tag="xp")
                if s < 4:
                    # borders only need zeroing once per pool buffer; the
                    # interior is fully rewritten every rotation
                    nc.vector.memset(xp[:, 0:1, :], 0.0)
                    nc.vector.memset(xp[:, HP - 1:HP, :], 0.0)
                    nc.vector.memset(xp[:, 1:HP - 1, 0:1], 0.0)
                    nc.vector.memset(xp[:, 1:HP - 1, WP - 1:WP], 0.0)
                for h in range(2):
                    ppt = pp_pool.tile([DIM, HALF], FP, tag="pp")
                    nc.tensor.matmul(ppt[:], lhsT=pw0,
                                     rhs=px_sb[s][:, XB + HW * h:
                                                  XB + HW * h + HALF],
                                     start=True, stop=False)
                    nc.tensor.matmul(ppt[:], lhsT=pw1,
                                     rhs=px_sb[s][:, XB + HW * h + HALF:
                                                  XB + HW * (h + 1)],
                                     start=False, stop=True)
                    dst = xp[:, 1 + HROWS * h:1 + HROWS * (h + 1), 1:1 + W]
                    src = ppt[:].rearrange("p (a b) -> p a b", b=W)
                    if h == 0:
                        # h0 on DVE, h1 on ACT: parallel evictions unblock
                        # the dependent conv two engine-ops sooner
                        nc.vector.tensor_scalar_add(dst, src, pb_ap)
                    else:
                        nc.scalar.activation(
                            dst, src,
                            mybir.ActivationFunctionType.Identity,
                            bias=pb_ap,
                        )
                return xp

            def wtap(s, t):
                return px_sb[s][:, t * DIM:(t + 1) * DIM]

            def conv(s, xp):
                o_sb = opool.tile([DIM, HW], BF, tag="o")
                pct0 = pc_pool.tile([DIM, HALF], FP, tag="pc")
                pct1 = pc_pool.tile([DIM, HALF], FP, tag="pc")
                pcts = [pct0, pct1]
                qb = qb_ap(s)
                # tap-outer: consecutive matmuls share the stationary weights
                for t in range(KK):
                    kh, kw = divmod(t, K)
                    for h in range(2):
                        nc.tensor.matmul(
                            pcts[h][:],
                            lhsT=wtap(s, t),
                            rhs=xp[:, HROWS * h + kh:HROWS * (h + 1) + kh,
                                   kw:kw + W],
                            start=(t == 0), stop=(t == KK - 1))
                # evict half 0 on DVE, half 1 on ACT; bf16 out halves traffic
                nc.vector.tensor_scalar_add(o_sb[:, 0:HALF], pct0[:], qb)
                nc.scalar.activation(
                    o_sb[:, HALF:HW], pct1[:],
                    mybir.ActivationFunctionType.Identity, bias=qb)
                nc.scalar.dma_start(out=out[s], in_=o_sb[:])

            def conv_last(s, xp):
                # h-outer + column-chunked second half: each chunk evicts and
                # stores while later chunks' taps still run -> short tail
                o_sb = opool.tile([DIM, HW], BF, tag="o")
                qb = qb_ap(s)
                pct0 = pc_pool.tile([DIM, HALF], FP, tag="pc")
                for t in range(KK):
                    kh, kw = divmod(t, K)
                    nc.tensor.matmul(
                        pct0[:], lhsT=wtap(s, t),
                        rhs=xp[:, kh:HROWS + kh, kw:kw + W],
                        start=(t == 0), stop=(t == KK - 1))
                nc.vector.tensor_scalar_add(o_sb[:, 0:HALF], pct0[:], qb)
                nc.scalar.dma_start(out=out[s, :, 0:HALF],
                                    in_=o_sb[:, 0:HALF])
                for c in range(2):
                    pcq = pc_pool.tile([DIM, HALF], FP, tag="pc")
                    r0 = HROWS + QROWS * c
                    c0 = HALF + QUART * c
                    for t in range(KK):
                        kh, kw = divmod(t, K)
                        nc.tensor.matmul(
                            pcq[:, 0:QUART], lhsT=wtap(s, t),
                            rhs=xp[:, r0 + kh:r0 + QROWS + kh, kw:kw + W],
                            start=(t == 0), stop=(t == KK - 1))
                    nc.vector.tensor_scalar_add(
                        o_sb[:, c0:c0 + QUART], pcq[:, 0:QUART], qb)
                    nc.scalar.dma_start(out=out[s, :, c0:c0 + QUART],
                                        in_=o_sb[:, c0:c0 + QUART])

            # software pipeline, depth-1 at the head: conv(0)'s deps (w0 on
            # the ACT ring + proj(0) evictions) are ready ~0.8us before x1's
            # DMA is visible, so conv(0) runs first and fills proj(1)'s
            # delivery wait; proj(2) then rebuilds the depth-2 pipeline that
            # hides eviction latency for the remaining samples
            xps = [proj(0)]
            conv(0, xps[0])
            xps.append(proj(1))
            for s in range(2, SPC):
                xps.append(proj(s))
                conv(s - 1, xps[s - 1])
            conv_last(SPC - 1, xps[SPC - 1])

    nc.compile()
    return nc


def _prep(question_rep, lhs_rep, rhs_rep, proj_w, proj_b):
    """Host-side shard + layout prep (reshape/transpose + bf16 cast)."""
    qr = np.ascontiguousarray(question_rep, dtype=np.float32)
    # conv weights: [B, o, i, kh, kw] -> [B, i, (kh kw), o] so each tap is a
    # ready lhsT [i, o] block
    qw = qr[:, :WDIM].reshape(B, DIM, DIM, K, K).transpose(0, 2, 3, 4, 1)
    qw = np.ascontiguousarray(qw).reshape(B, DIM, QWC)
    qb = np.ascontiguousarray(qr[:, WDIM:])             # [B, 128]
    xl = np.asarray(lhs_rep, dtype=np.float32).reshape(B, DIM, HW)
    xr = np.asarray(rhs_rep, dtype=np.float32).reshape(B, DIM, HW)
    pwt = np.asarray(proj_w, dtype=np.float32).T        # [256, 128]
    pw_h = np.concatenate([pwt[:DIM], pwt[DIM:]], axis=1).astype(BF_NP)
    pb = np.asarray(proj_b, dtype=np.float32).reshape(DIM, 1)

    in_maps = []
    for c in range(NCORES):
        sl = slice(c * SPC, (c + 1) * SPC)
        # biases ride as fp32 bit-patterns in bf16 columns (device bitcasts);
        # the whole const block replicates into every sample's px row so it
        # arrives inside sample 0's big first DMA (no tiny-descriptor DMA)
        biasm = np.concatenate([qb[sl].T, pb], axis=1).astype(np.float32)
        bias_bf = np.ascontiguousarray(biasm).view(np.uint16).view(BF_NP)
        cstm = np.concatenate([pw_h, bias_bf], axis=1)  # [128, 274] bf16
        cst8 = np.broadcast_to(cstm[None], (SPC, DIM, CSTC))
        pxm = np.concatenate(
            [qw[sl].astype(BF_NP), cst8,
             xl[sl, :, :HALF].astype(BF_NP), xr[sl, :, :HALF].astype(BF_NP),
             xl[sl, :, HALF:].astype(BF_NP), xr[sl, :, HALF:].astype(BF_NP)],
            axis=2)  # [SPC, 128, 3474]
        in_maps.append({
            "px": np.ascontiguousarray(pxm),
        })
    return in_maps


def kernel(question_rep, lhs_rep, rhs_rep, proj_w, proj_b, _run_kwargs=None):
    if "nc" not in _BUILT:
        _BUILT["nc"] = build_nc()
    nc = _BUILT["nc"]
    in_maps = _prep(question_rep, lhs_rep, rhs_rep, proj_w, proj_b)
    res = run_bass_kernel_spmd(nc, in_maps, core_ids=list(range(NCORES)),
                               **(_run_kwargs or {}))
    out = np.concatenate([np.asarray(res.results[c]["out"])
                          for c in range(NCORES)], axis=0)
    if _run_kwargs is not None:
        _BUILT["last_result"] = res
    return out.astype(np.float32).reshape(B, DIM, H, W)


if __name__ == "__main__":
    rng = np.random.default_rng(0)
    inputs = {
        "question_rep": rng.standard_normal((B, WDIM + DIM), dtype=np.float32) * 0.05,
        "lhs_rep": rng.standard_normal((B, DIM, H, W), dtype=np.float32),
        "rhs_rep": rng.standard_normal((B, DIM, H, W), dtype=np.float32),
        "proj_w": rng.standard_normal((DIM, 2 * DIM), dtype=np.float32),
        "proj_b": rng.standard_normal((DIM,), dtype=np.float32) * 0.01,
    }
    out = kernel(**inputs)
    print("ran, out shape:", out.shape)


# revision 27
# speedup vs baseline: 1.0277x; 1.0141x over previous
"""Dynamic per-sample CNN (nn_ConvFunc) Trainium2 Bass kernel — bf16 version.

Reference computation (per sample b):
  cnn_inp = proj_w @ cat(lhs, rhs) + proj_b          # 1x1 conv, [128, 32, 32]
  out     = conv3x3(cnn_inp, W_b) + bias_b           # W_b, bias_b unpacked from question_rep[b]

Sharding: pure data parallel, 8 samples per NeuronCore (batch 64 / 8 cores).

Design:
  - all matmul operands bf16 (tolerance 2e-2 >> bf16's ~3e-3): fp32 matmul
    streams at half rate on the trn2 PE, so bf16 halves PE time (90112
    columns -> 37.5us @ 2.4GHz) and halves HBM traffic.
  - per-sample inputs packed host-side into one [128, 3474] row
    ([qw | consts | xl_h0 | xr_h0 | xl_h1 | xr_h1]); loads split x-part /
    w-part and issued up-front on the sync HWDGE ring in exact consumption
    order (x0, x1, w0, x2, w1, ...) — each DMA_DIRECT2D issue costs ~650ns
    of sequencer time and completion semaphores lag the last byte by ~2us,
    so few big DMAs in consumption order win.
  - proj weights + biases ride inside sample 0's px row (biases as fp32
    bit-pattern pairs of bf16 columns, bitcast on device): no separate
    small-descriptor const DMA (a 36B/partition DMA starved behind the px
    stream cost 7us in an earlier rev).
  - dummy warmup matmuls fill the DMA ramp and known early stalls so the PE
    HAM clock gate lifts at ~11us and never re-throttles (idle >3.4us would
    drop the PE clock 2.4 -> 1.2 GHz).
  - PSUM eviction split across DVE (proj/conv h0) and ACT (proj/conv h1);
    output stored bf16 (host upcasts). Last sample runs h-outer with its
    second half column-chunked so the final evict+store tail is ~256 cols.
"""

import numpy as np
import ml_dtypes

import concourse.bass as bass
import concourse.mybir as mybir
from concourse import bacc
from concourse.tile import TileContext
from concourse.bass_utils import run_bass_kernel_spmd

# Problem shapes (hardcoded per contract)
B = 64
DIM = 128
H = W = 32
K = 3
KK = K * K
HW = H * W             # 1024
WDIM = DIM * DIM * KK  # 147456
NCORES = 8
SPC = B // NCORES      # samples per core
HP, WP = H + 2, W + 2  # padded 34x34
HALF = HW // 2         # 512 columns per PSUM bank
QUART = HALF // 2      # 256-col tail chunks
HROWS = H // 2         # 16 output rows per half
QROWS = HROWS // 2     # 8 rows per tail chunk
QWC = KK * DIM         # 1152 conv-weight cols per sample
CSTC = 2 * DIM + 2 * (SPC + 1)  # pw cols + bitcast fp32 bias cols (274)
XB = QWC + CSTC        # x columns start here
PXC = XB + 2 * HW      # 3474 packed cols: [qw | cst | xlh0 | xrh0 | xlh1 | xrh1]

FP = mybir.dt.float32
BF = mybir.dt.bfloat16
BF_NP = ml_dtypes.bfloat16

_BUILT = {}


def build_nc():
    nc = bacc.Bacc("TRN2", target_bir_lowering=False, debug=False,
                   num_devices=NCORES)

    px = nc.declare_dram_parameter("px", [SPC, DIM, PXC], BF, isOutput=False)
    out = nc.declare_dram_parameter("out", [SPC, DIM, HW], BF, isOutput=True)

    with TileContext(nc) as tc:
        with (
            tc.tile_pool(name="const", bufs=1) as cpool,
            tc.tile_pool(name="pxpool", bufs=SPC) as pxpool,
            tc.tile_pool(name="xppool", bufs=4) as xppool,
            tc.tile_pool(name="opool", bufs=4) as opool,
            tc.tile_pool(name="pp_pool", bufs=3, space="PSUM") as pp_pool,
            tc.tile_pool(name="pc_pool", bufs=5, space="PSUM") as pc_pool,
        ):
            warm = cpool.tile([DIM, HALF], BF)
            # gates the PE warmup matmuls; gpsimd reaches its first user
            # instruction earliest and is otherwise idle
            nc.gpsimd.memset(warm[:], 0.0)

            px_sb = [pxpool.tile([DIM, PXC], BF, tag="px", name=f"px{s}")
                     for s in range(SPC)]

            # ---- all loads up-front on the sync ring, consumption order ---
            def load_x(s):
                # sample 0 carries the consts: load them in the same DMA
                lo = QWC if s == 0 else XB
                nc.sync.dma_start(out=px_sb[s][:, lo:PXC],
                                  in_=px[s, :, lo:PXC])

            def load_w(s):
                nc.sync.dma_start(out=px_sb[s][:, 0:QWC], in_=px[s, :, 0:QWC])

            # consumption order: conv(s) needs w(s) right after proj(s+1)
            # needs x(s+1), so pair them w-then-x. Sample 0 splits at the
            # pixel-half boundary so proj(0) h0 gates on a smaller DMA;
            # sample 7 loads its whole row in one DMA (qw+dead cst+x) to
            # keep the total issue count unchanged (issue slots cost ~650ns
            # each and delay every later gate — the v5 lesson).
            # x0 as ONE DMA: with the depth-1 pipeline head, conv(0) (not
            # proj(1)) follows proj(0), so the finer px0 half-gates no longer
            # pay; the saved issue slot pulls x1's visibility before proj(1)
            # needs it
            load_x(0)
            # w0 rides the otherwise-idle ACT HWDGE ring: it transfers
            # concurrently with the sync stream, pulling the conv(0) and
            # x1 gates (the delivery-bound window) ~0.5us earlier
            nc.scalar.dma_start(out=px_sb[0][:, 0:QWC], in_=px[0, :, 0:QWC])
            load_x(1)
            for s in range(SPC - 3):
                if s > 0:
                    load_w(s)
                load_x(s + 2)
            load_w(SPC - 3)
            nc.sync.dma_start(out=px_sb[SPC - 1][:], in_=px[SPC - 1])
            load_w(SPC - 2)

            # ---- PE warmup: lift the HAM clock gate during the DMA ramp ---
            # the warmup accumulator shares the proj PSUM pool: warmups and
            # proj matmuls are all PE-ordered, so the WAR costs nothing
            wmb = pp_pool.tile([DIM, HALF], FP, tag="pp")

            def warmup(n):
                # N=256 quanta: finer granularity wastes less PE time when
                # the gating DMA completes mid-warmup
                for _ in range(n):
                    nc.tensor.matmul(wmb[:, 0:QUART], lhsT=warm[:, 0:DIM],
                                     rhs=warm[:, 0:QUART],
                                     start=True, stop=True)

            warmup(16)

            pw0 = px_sb[0][:, QWC:QWC + DIM]
            pw1 = px_sb[0][:, QWC + DIM:QWC + 2 * DIM]

            def qb_ap(s):
                o = QWC + 2 * DIM + 2 * s
                return px_sb[0][:, o:o + 2].bitcast(FP)

            pb_ap = px_sb[0][:, XB - 2:XB].bitcast(FP)

            def proj(s):
                xp = xppool.tile([DIM, HP, WP], BF, tag="xp")
                if s < 4:
                    # borders only need zeroing once per pool buffer; the
                    # interior is fully rewritten every rotation
                    nc.vector.memset(xp[:, 0:1, :], 0.0)
                    nc.vector.memset(xp[:, HP - 1:HP, :], 0.0)
                    nc.vector.memset(xp[:, 1:HP - 1, 0:1], 0.0)
                    nc.vector.memset(xp[:, 1:HP - 1, WP - 1:WP], 0.0)
                for h in range(2):
                    ppt = pp_pool.tile([DIM, HALF], FP, tag="pp")
                    nc.tensor.matmul(ppt[:], lhsT=pw0,
                                     rhs=px_sb[s][:, XB + HW * h:
                                                  XB + HW * h + HALF],
                                     start=True, stop=False)
                    nc.tensor.matmul(ppt[:], lhsT=pw1,
                                     rhs=px_sb[s][:, XB + HW * h + HALF:
                                                  XB + HW * (h + 1)],
                                     start=False, stop=True)
                    dst = xp[:, 1 + HROWS * h:1 + HROWS * (h + 1), 1:1 + W]
                    src = ppt[:].rearrange("p (a b) -> p a b", b=W)
                    if h == 0:
                        # h0 on DVE, h1 on ACT: parallel evictions unblock
                        # the dependent conv two engine-ops sooner
                        nc.vector.tensor_scalar_add(dst, src, pb_ap)
                    else:
                        nc.scalar.activation(
                            dst, src,
                            mybir.ActivationFunctionType.Identity,
                            bias=pb_ap,
                        )
                return xp

            def wtap(s, t):
                return px_sb[s][:, t * DIM:(t + 1) * DIM]

            def conv(s, xp):
                o_sb = opool.tile([DIM, HW], BF, tag="o")
                pct0 = pc_pool.tile([DIM, HALF], FP, tag="pc")
                pct1 = pc_pool.tile([DIM, HALF], FP, tag="pc")
                pcts = [pct0, pct1]
                qb = qb_ap(s)
                # tap-outer: consecutive matmuls share the stationary weights
                for t in range(KK):
                    kh, kw = divmod(t, K)
                    for h in range(2):
                        nc.tensor.matmul(
                            pcts[h][:],
                            lhsT=wtap(s, t),
                            rhs=xp[:, HROWS * h + kh:HROWS * (h + 1) + kh,
                                   kw:kw + W],
                            start=(t == 0), stop=(t == KK - 1))
                # evict half 0 on DVE, half 1 on ACT; bf16 out halves traffic
                nc.vector.tensor_scalar_add(o_sb[:, 0:HALF], pct0[:], qb)
                nc.scalar.activation(
                    o_sb[:, HALF:HW], pct1[:],
                    mybir.ActivationFunctionType.Identity, bias=qb)
                nc.scalar.dma_start(out=out[s], in_=o_sb[:])

            def conv_last(s, xp):
                # h-outer + column-chunked second half: each chunk evicts and
                # stores while later chunks' taps still run -> short tail
                o_sb = opool.tile([DIM, HW], BF, tag="o")
                qb = qb_ap(s)
                pct0 = pc_pool.tile([DIM, HALF], FP, tag="pc")
                for t in range(KK):
                    kh, kw = divmod(t, K)
                    nc.tensor.matmul(
                        pct0[:], lhsT=wtap(s, t),
                        rhs=xp[:, kh:HROWS + kh, kw:kw + W],
                        start=(t == 0), stop=(t == KK - 1))
                nc.vector.tensor_scalar_add(o_sb[:, 0:HALF], pct0[:], qb)
                nc.scalar.dma_start(out=out[s, :, 0:HALF],
                                    in_=o_sb[:, 0:HALF])
                for c in range(2):
                    pcq = pc_pool.tile([DIM, HALF], FP, tag="pc")
                    r0 = HROWS + QROWS * c
                    c0 = HALF + QUART * c
                    for t in range(KK):
                        kh, kw = divmod(t, K)
                        nc.tensor.matmul(
                            pcq[:, 0:QUART], lhsT=wtap(s, t),
                            rhs=xp[:, r0 + kh:r0 + QROWS + kh, kw:kw + W],
                            start=(t == 0), stop=(t == KK - 1))
                    nc.vector.tensor_scalar_add(
                        o_sb[:, c0:c0 + QUART], pcq[:, 0:QUART], qb)
                    nc.scalar.dma_start(out=out[s, :, c0:c0 + QUART],
                                        in_=o_sb[:, c0:c0 + QUART])

            # software pipeline, depth-1 at the head: conv(0)'s deps (w0 on
            # the ACT ring + proj(0) evictions) are ready ~0.8us before x1's
            # DMA is visible, so conv(0) runs first and fills proj(1)'s
            # delivery wait; proj(2) then rebuilds the depth-2 pipeline that
            # hides eviction latency for the remaining samples
            xps = [proj(0)]
            conv(0, xps[0])
            xps.append(proj(1))
            for s in range(2, SPC):
                xps.append(proj(s))
                conv(s - 1, xps[s - 1])
            conv_last(SPC - 1, xps[SPC - 1])

    nc.compile()
    return nc


def _prep(question_rep, lhs_rep, rhs_rep, proj_w, proj_b):
    """Host-side shard + layout prep (reshape/transpose + bf16 cast)."""
    qr = np.ascontiguousarray(question_rep, dtype=np.float32)
    # conv weights: [B, o, i, kh, kw] -> [B, i, (kh kw), o] so each tap is a
    # ready lhsT [i, o] block
    qw = qr[:, :WDIM].reshape(B, DIM, DIM, K, K).transpose(0, 2, 3, 4, 1)
    qw = np.ascontiguousarray(qw).reshape(B, DIM, QWC)
    qb = np.ascontiguousarray(qr[:, WDIM:])             # [B, 128]
    xl = np.asarray(lhs_rep, dtype=np.float32).reshape(B, DIM, HW)
    xr = np.asarray(rhs_rep, dtype=np.float32).reshape(B, DIM, HW)
    pwt = np.asarray(proj_w, dtype=np.float32).T        # [256, 128]
    pw_h = np.concatenate([pwt[:DIM], pwt[DIM:]], axis=1).astype(BF_NP)
    pb = np.asarray(proj_b, dtype=np.float32).reshape(DIM, 1)

    in_maps = []
    for c in range(NCORES):
        sl = slice(c * SPC, (c + 1) * SPC)
        # biases ride as fp32 bit-patterns in bf16 columns (device bitcasts);
        # the whole const block replicates into every sample's px row so it
        # arrives inside sample 0's big first DMA (no tiny-descriptor DMA)
        biasm = np.concatenate([qb[sl].T, pb], axis=1).astype(np.float32)
        bias_bf = np.ascontiguousarray(biasm).view(np.uint16).view(BF_NP)
        cstm = np.concatenate([pw_h, bias_bf], axis=1)  # [128, 274] bf16
        cst8 = np.broadcast_to(cstm[None], (SPC, DIM, CSTC))
        pxm = np.concatenate(
            [qw[sl].astype(BF_NP), cst8,
             xl[sl, :, :HALF].astype(BF_NP), xr[sl, :, :HALF].astype(BF_NP),
             xl[sl, :, HALF:].astype(BF_NP), xr[sl, :, HALF:].astype(BF_NP)],
            axis=2)  # [SPC, 128, 3474]
        in_maps.append({
            "px": np.ascontiguousarray(pxm),
        })
    return in_maps


def kernel(question_rep, lhs_rep, rhs_rep, proj_w, proj_b, _run_kwargs=None):
    if "nc" not in _BUILT:
        _BUILT["nc"] = build_nc()
    nc = _BUILT["nc"]
    in_maps = _prep(question_rep, lhs_rep, rhs_rep, proj_w, proj_b)
    res = run_bass_kernel_spmd(nc, in_maps, core_ids=list(range(NCORES)),
                               **(_run_kwargs or {}))
    out = np.concatenate([np.asarray(res.results[c]["out"])
                          for c in range(NCORES)], axis=0)
    if _run_kwargs is not None:
        _BUILT["last_result"] = res
    return out.astype(np.float32).reshape(B, DIM, H, W)


if __name__ == "__main__":
    rng = np.random.default_rng(0)
    inputs = {
        "question_rep": rng.standard_normal((B, WDIM + DIM), dtype=np.float32) * 0.05,
        "lhs_rep": rng.standard_normal((B, DIM, H, W), dtype=np.float32),
        "rhs_rep": rng.standard_normal((B, DIM, H, W), dtype=np.float32),
        "proj_w": rng.standard_normal((DIM, 2 * DIM), dtype=np.float32),
        "proj_b": rng.standard_normal((DIM,), dtype=np.float32) * 0.01,
    }
    out = kernel(**inputs)
    print("ran, out shape:", out.shape)
